# revision 5
# baseline (speedup 1.0000x reference)
"""Trainium2 Bass kernel for nn_LinearTriParser (B=2,S=128,H=1024,A=256,C=14).

Math: score[b,i,j,k,c] = sh0[i,c]+st0[j,c]+sm0[k,c]; softmax over k with
mask k in [i,j]. Since sh0+st0 are constant in k, alpha only depends on sm0:
  valid (i<=j): alpha = exp(sm0[k])/sum_{k'=i..j} exp(sm0[k'])
  invalid (i>j): all scores masked => alpha uniform = 1/S
final[b,i,j,c] = sh1[i,c]+st1[j,c]+uni[c] + sum_k alpha*sm1[k,c]
With prefix sums P0=cumsum(exp(sm0)), P1=cumsum(exp(sm0)*sm1) over k:
  valid:   attn = (P1[j]-P1[i-1])/(P0[j]-P0[i-1])
  invalid: attn = mean_k(sm1)
The cubic tensor never materializes: per (b,i,j,c) it's two prefix-sum
lookups. Implemented as K=15 matmuls (14 "comb" delta rows broadcasting
i-indexed values + 1 row broadcasting j-indexed values) into [i, (j,c)]
tiles, then a masked divide.

Sharding: 8 cores x (batch b, j-quarter). Each core runs an identical
program; per-core behavior comes only from input data (its batch's rows
first in `memx`, per-core mask/jsel constants) and host-side reassembly.
"""

import numpy as np

B, S, H, A, C = 2, 128, 1024, 256, 14
P = 128
JW = 32            # j columns per core
W = JW * C         # 448 free width of cubic tiles
NB = 256           # B*S rows

F32 = None  # set after mybir import


def _build():
    import concourse.mybir as mybir
    import concourse.tile as tile
    from concourse import bacc

    f32 = mybir.dt.float32
    nc = bacc.Bacc("TRN2", target_bir_lowering=False, debug=False,
                   enable_asserts=False, num_devices=8)

    def din(name, shape):
        return nc.dram_tensor(name, shape, f32, kind="ExternalInput")

    memx = din("memx", [NB, H])
    Ws1 = {br: din(f"{br}W1", [H, A]) for br in "htm"}
    Bs1 = {br: din(f"{br}B1", [A]) for br in "htm"}
    Ws2 = {br: din(f"{br}W2", [A, A]) for br in "htm"}
    Bs2 = {br: din(f"{br}B2", [A]) for br in "htm"}
    sW = {nm: din(f"s{nm}W", [A, C]) for nm in ("0m", "1h", "1t", "1m")}
    sB = {nm: din(f"s{nm}B", [C]) for nm in ("0m", "1h", "1t", "1m")}
    uni = din("uni", [C])
    ident = din("ident", [P, P])
    comb = din("comb", [C, W])
    mask = din("mask", [P, W])
    invmask = din("invmask", [P, W])
    jsel = din("jsel", [P, JW])
    onesneg = din("onesneg", [2, P])
    outp = nc.dram_tensor("outp", [P, W], f32, kind="ExternalOutput")

    with tile.TileContext(nc) as tc:
        with (
            tc.tile_pool(name="pers", bufs=1) as pers,
            tc.tile_pool(name="work", bufs=3) as work,
            tc.tile_pool(name="ps_t", bufs=2, space="PSUM") as ps_t,
            tc.tile_pool(name="ps_mm", bufs=2, space="PSUM") as ps_mm,
            tc.tile_pool(name="ps_s", bufs=2, space="PSUM") as ps_s,
            tc.tile_pool(name="ps_c", bufs=2, space="PSUM") as ps_c,
        ):
            # ---- load constants / weights ----
            mem_sb = [pers.tile([P, H], f32, name=f"mem{t}", tag=f"mem{t}") for t in range(2)]
            for t in range(2):
                nc.sync.dma_start(mem_sb[t][:], memx.ap()[t * P:(t + 1) * P, :])
            w1_sb = {}
            w2_sb = {}
            b1_sb = {}
            b2_sb = {}
            for br in "htm":
                w1_sb[br] = pers.tile([P, 8 * A], f32, name=f"w1{br}", tag=f"w1{br}")
                nc.sync.dma_start(
                    w1_sb[br][:].rearrange("p (k a) -> p k a", k=8),
                    Ws1[br].ap().rearrange("(k p) a -> p k a", p=P),
                )
                w2_sb[br] = pers.tile([P, 2 * A], f32, name=f"w2{br}", tag=f"w2{br}")
                nc.sync.dma_start(
                    w2_sb[br][:].rearrange("p (k a) -> p k a", k=2),
                    Ws2[br].ap().rearrange("(k p) a -> p k a", p=P),
                )
                b1_sb[br] = pers.tile([P, 2], f32, name=f"b1{br}", tag=f"b1{br}")
                nc.sync.dma_start(
                    b1_sb[br][:], Bs1[br].ap().rearrange("(k p) -> p k", p=P))
                b2_sb[br] = pers.tile([P, 2], f32, name=f"b2{br}", tag=f"b2{br}")
                nc.sync.dma_start(
                    b2_sb[br][:], Bs2[br].ap().rearrange("(k p) -> p k", p=P))
            sw_sb = {}
            sb_sb = {}
            for nm in ("0m", "1h", "1t", "1m"):
                sw_sb[nm] = pers.tile([P, 2 * C], f32, name=f"sw{nm}", tag=f"sw{nm}")
                nc.sync.dma_start(
                    sw_sb[nm][:].rearrange("p (k a) -> p k a", k=2),
                    sW[nm].ap().rearrange("(k p) a -> p k a", p=P),
                )
                sb_sb[nm] = pers.tile([C, 1], f32, name=f"sb{nm}", tag=f"sb{nm}")
                nc.sync.dma_start(
                    sb_sb[nm][:], sB[nm].ap().rearrange("(k p) -> p k", p=C))
            uni_sb = pers.tile([C, 1], f32, name="uni", tag="uni")
            nc.sync.dma_start(uni_sb[:], uni.ap().rearrange("(k p) -> p k", p=C))
            id_sb = pers.tile([P, P], f32, name="ident", tag="ident")
            nc.sync.dma_start(id_sb[:], ident.ap())
            comb_sb = pers.tile([C, W], f32, name="comb", tag="comb")
            nc.sync.dma_start(comb_sb[:], comb.ap())
            mask_sb = pers.tile([P, W], f32, name="mask", tag="mask")
            nc.sync.dma_start(mask_sb[:], mask.ap())
            imask_sb = pers.tile([P, W], f32, name="imask", tag="imask")
            nc.sync.dma_start(imask_sb[:], invmask.ap())
            jsel_sb = pers.tile([P, JW], f32, name="jsel", tag="jsel")
            nc.sync.dma_start(jsel_sb[:], jsel.ap())
            on_sb = pers.tile([2, P], f32, name="on", tag="on")
            nc.sync.dma_start(on_sb[:], onesneg.ap())

            # ---- transpose X: [256,1024] -> 8 tiles [128(h), 256(bs)] ----
            xt = [pers.tile([P, NB], f32, name=f"xt{k}", tag=f"xt{k}") for k in range(8)]
            for k in range(8):
                for t in range(2):
                    pt = ps_t.tile([P, P], f32, name="ptr", tag="ptr")
                    nc.tensor.transpose(
                        pt[:], mem_sb[t][:, k * P:(k + 1) * P], id_sb[:])
                    nc.vector.tensor_copy(xt[k][:, t * P:(t + 1) * P], pt[:])

            # ---- 3 branch MLPs (transposed activations [A, 256]) ----
            hT = {}
            for br in "htm":
                a1 = [work.tile([P, NB], f32, name=f"a1_{m}", tag=f"a1_{m}") for m in range(2)]
                for m in range(2):
                    p1 = ps_mm.tile([P, NB], f32, name="p1", tag="pmm")
                    for k in range(8):
                        nc.tensor.matmul(
                            p1[:],
                            w1_sb[br][:, k * A + m * P: k * A + m * P + P],
                            xt[k][:],
                            start=(k == 0), stop=(k == 7),
                        )
                    nc.scalar.activation(
                        a1[m][:], p1[:], mybir.ActivationFunctionType.Relu,
                        bias=b1_sb[br][:, m:m + 1], scale=1.0)
                h2 = [pers.tile([P, NB], f32, name=f"h2{br}{m}", tag=f"h2{br}{m}") for m in range(2)]
                for m2 in range(2):
                    p2 = ps_mm.tile([P, NB], f32, name="p2", tag="pmm")
                    for k2 in range(2):
                        nc.tensor.matmul(
                            p2[:],
                            w2_sb[br][:, k2 * A + m2 * P: k2 * A + m2 * P + P],
                            a1[k2][:],
                            start=(k2 == 0), stop=(k2 == 1),
                        )
                    nc.scalar.activation(
                        h2[m2][:], p2[:], mybir.ActivationFunctionType.Identity,
                        bias=b2_sb[br][:, m2:m2 + 1], scale=1.0)
                hT[br] = h2

            # ---- score heads: sT[nm] = sW.T @ hT + b : [14, 256] ----
            sT = {}
            for nm, br in (("0m", "m"), ("1h", "h"), ("1t", "t"), ("1m", "m")):
                pS = ps_s.tile([C, NB], f32, name="pS", tag="psm")
                for k2 in range(2):
                    nc.tensor.matmul(
                        pS[:], sw_sb[nm][:, k2 * C:(k2 + 1) * C], hT[br][k2][:],
                        start=(k2 == 0), stop=(k2 == 1))
                sT[nm] = pers.tile([C, NB], f32, name=f"sT{nm}", tag=f"sT{nm}")
                nc.scalar.activation(
                    sT[nm][:], pS[:], mybir.ActivationFunctionType.Identity,
                    bias=sb_sb[nm][:], scale=1.0)

            # ---- prefix-sum softmax machinery (my batch = cols 0:128) ----
            sm0 = sT["0m"][:, 0:P]
            sm1 = sT["1m"][:, 0:P]
            sh1 = sT["1h"][:, 0:P]
            st1 = sT["1t"][:, 0:P]

            mx = work.tile([C, 1], f32, name="mx", tag="mx")
            nc.vector.tensor_reduce(mx[:], sm0, axis=mybir.AxisListType.X,
                                    op=mybir.AluOpType.max)
            nmx = work.tile([C, 1], f32, name="nmx", tag="nmx")
            nc.vector.tensor_scalar_mul(nmx[:], mx[:], -1.0)
            eE = work.tile([C, P], f32, name="eE", tag="eE")
            nc.scalar.activation(eE[:], sm0, mybir.ActivationFunctionType.Exp,
                                 bias=nmx[:], scale=1.0)
            eS = work.tile([C, P], f32, name="eS", tag="eS")
            nc.vector.tensor_mul(eS[:], eE[:], sm1)
            ssum = work.tile([C, 1], f32, name="ssum", tag="ssum")
            nc.vector.tensor_reduce(ssum[:], sm1, axis=mybir.AxisListType.X,
                                    op=mybir.AluOpType.add)
            meanc = work.tile([C, 1], f32, name="meanc", tag="meanc")
            nc.vector.tensor_scalar_mul(meanc[:], ssum[:], 1.0 / P)

            p0 = work.tile([C, P], f32, name="p0", tag="p0")
            nc.vector.tensor_tensor_scan(
                p0[:], eE[:], eE[:], 0.0,
                op0=mybir.AluOpType.add, op1=mybir.AluOpType.bypass)
            p1c = work.tile([C, P], f32, name="p1c", tag="p1c")
            nc.vector.tensor_tensor_scan(
                p1c[:], eS[:], eS[:], 0.0,
                op0=mybir.AluOpType.add, op1=mybir.AluOpType.bypass)
            # nP1p = meanc*P0 - P1  (= -P1')
            np1p = work.tile([C, P], f32, name="np1p", tag="np1p")
            nc.vector.scalar_tensor_tensor(
                np1p[:], p0[:], meanc[:], p1c[:],
                op0=mybir.AluOpType.mult, op1=mybir.AluOpType.subtract)

            # shifts (prepend 0): Z0 = P0[i-1], Z1 = nP1p[i-1]
            z0 = work.tile([C, P], f32, name="z0", tag="z0")
            nc.vector.memset(z0[:, 0:1], 0.0)
            nc.vector.tensor_copy(z0[:, 1:P], p0[:, 0:P - 1])
            nz0 = work.tile([C, P], f32, name="nz0", tag="nz0")
            nc.vector.tensor_scalar_mul(nz0[:], z0[:], -1.0)
            z1 = work.tile([C, P], f32, name="z1", tag="z1")
            nc.vector.memset(z1[:, 0:1], 0.0)
            nc.vector.tensor_copy(z1[:, 1:P], np1p[:, 0:P - 1])

            # sh1' = sh1 + uni + meanc
            uadd = work.tile([C, 1], f32, name="uadd", tag="uadd")
            nc.vector.tensor_add(uadd[:], uni_sb[:], meanc[:])
            sh1p = work.tile([C, P], f32, name="sh1p", tag="sh1p")
            nc.vector.tensor_scalar_add(sh1p[:], sh1, uadd[:])

            # transpose P0 | nP1p | st1 -> [128, 42]
            pT3 = ps_s.tile([P, 3 * C], f32, name="pT3", tag="psm")
            for ci, src in enumerate((p0[:], np1p[:], st1)):
                nc.tensor.transpose(pT3[:, ci * C:(ci + 1) * C], src,
                                    id_sb[0:C, 0:C])
            t3 = work.tile([P, 3 * C], f32, name="t3", tag="t3")
            nc.vector.tensor_copy(t3[:], pT3[:])
            # select this core's 32 j rows: [32, 42]
            pj = ps_s.tile([JW, 3 * C], f32, name="pj", tag="psm")
            nc.tensor.matmul(pj[:], jsel_sb[:], t3[:], start=True, stop=True)
            j3 = work.tile([JW, 3 * C], f32, name="j3", tag="j3")
            nc.vector.tensor_copy(j3[:], pj[:])

            # rhs tiles [15, 448]: rows 0:14 comb, row 14 flatten(j3 part)
            rhs = {}
            for ci, nm in enumerate(("d", "n", "b")):
                r = pers.tile([15, W], f32, name=f"rhs{nm}", tag=f"rhs{nm}")
                nc.vector.tensor_copy(r[0:C, :], comb_sb[:])
                nc.sync.dma_start(
                    r[14:15, :].rearrange("p (a b) -> p a b", a=JW),
                    j3[0:JW, ci * C:(ci + 1) * C],
                )
                rhs[nm] = r

            # lhsT tiles [15, 128]
            lb = pers.tile([15, P], f32, name="lb", tag="lb")
            nc.vector.tensor_copy(lb[0:C, :], sh1p[:])
            nc.sync.dma_start(lb[14:15, :], onesneg.ap()[0:1, :])
            ld = pers.tile([15, P], f32, name="ld", tag="ld")
            nc.vector.tensor_copy(ld[0:C, :], nz0[:])
            nc.sync.dma_start(ld[14:15, :], onesneg.ap()[0:1, :])
            ln = pers.tile([15, P], f32, name="ln", tag="ln")
            nc.vector.tensor_copy(ln[0:C, :], z1[:])
            nc.sync.dma_start(ln[14:15, :], onesneg.ap()[1:2, :])

            # cubic matmuls [128, 448]
            pB = ps_c.tile([P, W], f32, name="pB", tag="pc")
            nc.tensor.matmul(pB[:], lb[:], rhs["b"][:], start=True, stop=True)
            pD = ps_c.tile([P, W], f32, name="pD", tag="pc")
            nc.tensor.matmul(pD[:], ld[:], rhs["d"][:], start=True, stop=True)
            pN = ps_c.tile([P, W], f32, name="pN", tag="pc")
            nc.tensor.matmul(pN[:], ln[:], rhs["n"][:], start=True, stop=True)

            # masked divide + final add
            nM = work.tile([P, W], f32, name="nM", tag="nM")
            nc.vector.tensor_mul(nM[:], pN[:], mask_sb[:])
            dm = work.tile([P, W], f32, name="dm", tag="dm")
            nc.vector.tensor_mul(dm[:], pD[:], mask_sb[:])
            dsafe = work.tile([P, W], f32, name="dsafe", tag="dsafe")
            nc.vector.tensor_add(dsafe[:], dm[:], imask_sb[:])
            rec = work.tile([P, W], f32, name="rec", tag="rec")
            nc.vector.reciprocal(rec[:], dsafe[:])
            at = work.tile([P, W], f32, name="at", tag="at")
            nc.vector.tensor_mul(at[:], nM[:], rec[:])
            fin = work.tile([P, W], f32, name="fin", tag="fin")
            nc.vector.tensor_add(fin[:], pB[:], at[:])
            nc.sync.dma_start(outp.ap(), fin[:])

    nc.finalize()
    return nc


_NC_CACHE = None


def kernel(**inputs):
    from concourse.bass_utils import run_bass_kernel_spmd

    global _NC_CACHE
    if _NC_CACHE is None:
        _NC_CACHE = _build()
    nc = _NC_CACHE

    memory = np.asarray(inputs["memory"], dtype=np.float32)

    # host-side per-core constants (index/selection only)
    comb = (np.arange(C)[:, None, None] ==
            np.arange(C)[None, None, :]).astype(np.float32)
    comb = np.broadcast_to(comb, (C, JW, C)).reshape(C, W).copy()
    ident = np.eye(P, dtype=np.float32)

    common = {
        "ident": ident, "comb": comb,
        "onesneg": np.stack([np.ones(P, np.float32), -np.ones(P, np.float32)]), "uni": np.asarray(inputs["uni"], np.float32),
    }
    for br in "htm":
        common[f"{br}W1"] = np.asarray(inputs[f"{br}_W1"], np.float32)
        common[f"{br}B1"] = np.asarray(inputs[f"{br}_b1"], np.float32)
        common[f"{br}W2"] = np.asarray(inputs[f"{br}_W2"], np.float32)
        common[f"{br}B2"] = np.asarray(inputs[f"{br}_b2"], np.float32)
    for nm in ("0m", "1h", "1t", "1m"):
        br = nm[1]
        common[f"s{nm}W"] = np.asarray(inputs[f"s{nm[0]}{br}_W"], np.float32)
        common[f"s{nm}B"] = np.asarray(inputs[f"s{nm[0]}{br}_b"], np.float32)

    in_maps = []
    ii = np.arange(P)[:, None]
    for cid in range(8):
        b, jq = cid // 4, cid % 4
        j0 = jq * JW
        jg = j0 + np.arange(JW)
        m = (jg[None, :, None] >= ii[:, :, None]).astype(np.float32)
        m = np.broadcast_to(m, (P, JW, C)).reshape(P, W).copy()
        js = np.zeros((P, JW), np.float32)
        js[j0 + np.arange(JW), np.arange(JW)] = 1.0
        memx = np.concatenate([memory[b], memory[1 - b]], axis=0)
        in_maps.append({
            **common,
            "memx": np.ascontiguousarray(memx),
            "mask": m, "invmask": (1.0 - m), "jsel": js,
        })

    global _LAST_IN_MAPS
    _LAST_IN_MAPS = in_maps
    res = run_bass_kernel_spmd(nc, in_maps, core_ids=list(range(8)))
    out = np.zeros((B, S, S, C), dtype=np.float32)
    for cid in range(8):
        b, jq = cid // 4, cid % 4
        j0 = jq * JW
        out[b, :, j0:j0 + JW, :] = res.results[cid]["outp"].reshape(P, JW, C)
    return out
